# revision 55
# baseline (speedup 1.0000x reference)
"""Multi-head attention block (dense transformer) on 8 trn2 NeuronCores.

Sharding: batch (4) x head-group (2 groups of 8 heads) = 8 cores. Each core
computes, for its batch b and its 8 heads:
    qkv slice -> per-head softmax(q k^T / sqrt(D)) v -> partial out proj.
Host sums the two head-group partials per batch and adds the output bias.

v2 dataflow (vs the bf16 v1 baseline):
  * QK and V projections run as fp8e4 DoubleRow matmuls with a 3-term
    hi/lo split (Whi.Xhi + Whi.Xlo + Wlo.Xhi), operands pre-split on the
    host; 2-2.7x fewer PE cycles than bf16 at ~bf16 accuracy.
  * S^T = k q^T runs as fp8 DoubleRow with each head's 64 contraction dims
    laid out as 32 partitions x 2 interleave slots, halving PE rows.
    q/k stay unscaled (~N(0,1), fp8 sweet spot); the 1/sqrt(D) logit scale
    is folded into the exp.
  * softmax exp splits between the ACT engine (direct from PSUM) and the
    DVE engine, which computes exp(x) ~= (1+x/16)^16 in fp16 via 4
    squarings -- balances the two engine streams.
  * P@V runs transposed (lhsT = P tile, rhs = V[keys, 64+ones]) so the
    output lands as [queries, dims]: full 128-partition output rows, half
    the PE cost of the v1 orientation, and the softmax denominator
    becomes a per-partition scalar (no DRAM broadcast bounce). A PE
    transpose then restores [dims, queries] for the bf16 out projection.
"""

import numpy as np
import ml_dtypes
import jax
import jax.core
from jax.experimental.shard_map import shard_map
from jax.sharding import Mesh, PartitionSpec

import concourse.bass as bass
import concourse.mybir as mybir
import concourse.tile as tile
import concourse.bass2jax as bass2jax
from concourse import masks
from concourse.vector_clock import ScopedClock

# ---------------------------------------------------------------------------
# Workaround for the pinned walrus compiler: it rejects instructions carrying
# more than one sync wait. Split extra waits onto NOPs inserted immediately
# before the instruction in the same engine stream (identical semantics: the
# engine blocks on each wait in turn).
# ---------------------------------------------------------------------------
_MAX_WAITS = 1
_patched = False


def _split_waits(ordered):
    for bb_name, insts in ordered.items():
        out = []
        for inst in insts:
            si = inst.sync_info
            waits = list(si.on_wait) if si and si.on_wait else []
            if len(waits) > _MAX_WAITS:
                rest, keep = waits[:-_MAX_WAITS], waits[-_MAX_WAITS:]
                for k in range(0, len(rest), _MAX_WAITS):
                    out.append(mybir.InstNoOp(
                        name=f"{inst.name}-wsplit{k}",
                        sync_info=mybir.SyncInfo(
                            on_wait=rest[k:k + _MAX_WAITS], on_update=[]),
                        bass_nofuse=True,
                        engine=inst.engine,
                    ))
                inst.sync_info = mybir.SyncInfo(
                    on_wait=keep, on_update=list(si.on_update or []))
            out.append(inst)
        ordered[bb_name] = out
    return ordered


def _install_patches():
    global _patched
    if _patched:
        return
    _patched = True

    orig_lower = tile.TileContext._lower_ordered_insts

    def lower_with_split(self, ordered):
        return orig_lower(self, _split_waits(ordered))

    tile.TileContext._lower_ordered_insts = lower_with_split

    def drain_and_barrier(self, tick_clock, wait_clock):
        nc = self.nc
        drain_inst = nc.sync.drain()
        wait_clock.add_sem_waits(
            drain_inst.ins, ScopedClock({None: tick_clock.global_clock}))
        si = drain_inst.ins.sync_info
        waits = list(si.on_wait) if si and si.on_wait else []
        upds = list(si.on_update) if si and si.on_update else []
        if len(waits) > _MAX_WAITS:
            drain_inst.ins.sync_info = mybir.SyncInfo(
                on_wait=waits[:_MAX_WAITS], on_update=upds)
            for i in range(_MAX_WAITS, len(waits), _MAX_WAITS):
                nop = nc.sync.nop()
                nop.ins.sync_info = mybir.SyncInfo(
                    on_wait=waits[i:i + _MAX_WAITS], on_update=[])
        nc.all_engine_barrier()
        assert self.sems is not None
        popped = nc._tile_sem_poison_stack.pop()
        assert popped is self._sem_poison
        nc.clear_and_free_semaphores(list(self.sems.allocated().values()))
        nc.all_engine_barrier()

    tile.TileContext._drain_and_barrier = drain_and_barrier


# ---------------------------------------------------------------------------
# Problem constants (hardcoded per the task contract).
# ---------------------------------------------------------------------------
B, N, D, H, HD = 4, 2048, 1024, 16, 64
NCORES = 8
HPC = 8                 # heads per core
NPAIRS = HPC // 2       # head pairs per core
KT = 4                  # DoubleRow contraction tiles (256 dims each) over D
NJ = N // 128           # 16 key tiles
NIC = N // 512          # 4 query chunks of 512
SCALE = float(D) ** -0.5
WSCALE = 32.0           # host scales q/k/v weights by 32 (fp8 range); the
                        # logit scale absorbs 1/32^2, w_out absorbs 1/32
EXP_SCALE = SCALE / (WSCALE * WSCALE)
EXP_N = 16              # DVE ladder: exp(x) ~= (1 + x/EXP_N)^EXP_N

BF16 = mybir.dt.bfloat16
F16 = mybir.dt.float16
F32 = mybir.dt.float32
F8 = mybir.dt.float8e4
U8 = mybir.dt.uint8
FT = mybir.ActivationFunctionType
DR = mybir.MatmulPerfMode.DoubleRow
MUL = mybir.AluOpType.mult
ADD = mybir.AluOpType.add


def _u8(ap):
    return ap.bitcast(U8).rearrange("p a b -> p (a b)")


def build_nc(loop_n: int = 1, dbg: bool = False) -> bass.Bass:
    """loop_n > 1 wraps the whole body in a hardware loop (benchmark builds
    only) so per-iteration device time can be extracted from wall clock."""
    _install_patches()
    nc = bass.Bass()
    if dbg:
        d_qp = nc.dram_tensor("d_qp", [64, 2 * N], U8, kind="ExternalOutput")
        d_kp = nc.dram_tensor("d_kp", [64, 2 * N], U8, kind="ExternalOutput")
        d_vp = nc.dram_tensor("d_vp", [128, HPC * (HD + 1)], F32, kind="ExternalOutput")
        d_ut = nc.dram_tensor("d_ut", [128, 1024], F32, kind="ExternalOutput")
        d_ot = nc.dram_tensor("d_ot", [128, N], F32, kind="ExternalOutput")
        d_st2 = nc.dram_tensor("d_st2", [128, 128], F32, kind="ExternalOutput")
        d_rt = nc.dram_tensor("d_rt", [128, 4], F32, kind="ExternalOutput")

    xth = nc.dram_tensor("xth", [4 * 128, 2 * N], U8, kind="ExternalInput")
    xtl = nc.dram_tensor("xtl", [4 * 128, 2 * N], U8, kind="ExternalInput")
    wqh = nc.dram_tensor("wqh", [4 * 128, 2 * 1024], U8, kind="ExternalInput")
    wql = nc.dram_tensor("wql", [4 * 128, 2 * 1024], U8, kind="ExternalInput")
    # pair-0 k/q weight columns prepacked: [k 512:640 | q 0:128] per slot
    w0h = nc.dram_tensor("w0h", [4 * 128, 2 * 256], U8, kind="ExternalInput")
    w0l = nc.dram_tensor("w0l", [4 * 128, 2 * 256], U8, kind="ExternalInput")
    wvh = nc.dram_tensor("wvh", [4 * 128, 2 * 512], U8, kind="ExternalInput")
    wvl = nc.dram_tensor("wvl", [4 * 128, 2 * 512], U8, kind="ExternalInput")
    wo = nc.dram_tensor("wo", [512, D], BF16, kind="ExternalInput")
    out = nc.dram_tensor("out", [N, D], BF16, kind="ExternalOutput")

    import contextlib

    with tile.TileContext(nc) as tc:
        loop_ctx = (tc.For_i(0, loop_n, 1,
                             hint_engines=(mybir.EngineType.PE,
                                           mybir.EngineType.Activation,
                                           mybir.EngineType.DVE,
                                           mybir.EngineType.SP))
                    if loop_n > 1 else contextlib.nullcontext())
        with (
            loop_ctx,
            tc.tile_pool(name="persist", bufs=1) as pers,
            tc.tile_pool(name="expp", bufs=2, space="PSUM") as expp,
            tc.tile_pool(name="mmp", bufs=2, space="PSUM") as mmp,
            tc.tile_pool(name="pvp", bufs=2, space="PSUM") as pvp,
            tc.tile_pool(name="utp", bufs=20) as utp,
            tc.tile_pool(name="st2p", bufs=12) as st2p,
            tc.tile_pool(name="rtp", bufs=4) as rtp,
            tc.tile_pool(name="fsp", bufs=4) as fsp,
            tc.tile_pool(name="dbgp", bufs=1) as dbgp,
        ):
            # ---- persistent SBUF tensors -----------------------------------
            x8h = [pers.tile([128, 2, N], F8, tag=f"x8h{i}", name=f"x8h{i}") for i in range(KT)]
            x8l = [pers.tile([128, 2, N], F8, tag=f"x8l{i}", name=f"x8l{i}") for i in range(KT)]
            w8h = [pers.tile([128, 2, 1024], F8, tag=f"w8h{i}", name=f"w8h{i}") for i in range(KT)]
            w8l = [pers.tile([128, 2, 1024], F8, tag=f"w8l{i}", name=f"w8l{i}") for i in range(KT)]
            w08h = [pers.tile([128, 2, 256], F8, tag=f"w08h{i}", name=f"w08h{i}") for i in range(KT)]
            w08l = [pers.tile([128, 2, 256], F8, tag=f"w08l{i}", name=f"w08l{i}") for i in range(KT)]
            v8h = [pers.tile([128, 2, 512], F8, tag=f"v8h{i}", name=f"v8h{i}") for i in range(KT)]
            v8l = [pers.tile([128, 2, 512], F8, tag=f"v8l{i}", name=f"v8l{i}") for i in range(KT)]
            wo_sb = [pers.tile([128, D], BF16, tag=f"wo{i}", name=f"wo{i}") for i in range(4)]
            qp = [pers.tile([64, 2, N], F8, tag=f"qp{i}", name=f"qp{i}") for i in range(NPAIRS)]
            kp = [pers.tile([64, 2, N], F8, tag=f"kp{i}", name=f"kp{i}") for i in range(NPAIRS)]
            vp = [pers.tile([128, HPC, HD + 1], BF16, tag=f"vp{i}", name=f"vp{i}")
                  for i in range(NJ)]
            ot = [pers.tile([128, N], BF16, tag=f"ot{i}", name=f"ot{i}") for i in range(NPAIRS)]
            ident = pers.tile([128, 128], BF16, tag="ident", name="ident")
            zt260 = pers.tile([128, NIC * (HD + 1)], BF16, tag="zt260", name="zt260")

            def load2(dst_ap, src, i):
                """Split a [128, C]-byte row-block load into column halves so
                the transfers land on two DMA queues."""
                c = src.shape[1]
                h = c // 2
                nc.sync.dma_start(out=dst_ap[:, 0:h],
                                  in_=src[i * 128:(i + 1) * 128, 0:h])
                nc.sync.dma_start(out=dst_ap[:, h:c],
                                  in_=src[i * 128:(i + 1) * 128, h:c])

            # DMA order tuned for pipeline start-up. Column-sliced loads:
            # hp0's k/q weight slices and x's ic0 chunk land first so the
            # first projections start early instead of waiting for the full
            # 8MB input load. Each slice is one DMA whose 3D access pattern
            # covers both interleave slots, keeping the SP issue count low.
            def loadw(c0, c1):
                for i in range(KT):
                    for (dst, src) in ((w8h, wqh), (w8l, wql)):
                        nc.sync.dma_start(
                            out=dst[i][:, :, c0:c1].bitcast(U8),
                            in_=src[i * 128:(i + 1) * 128, :]
                            .rearrange("p (s n) -> p s n", s=2)[:, :, c0:c1])

            def loadx(ic_):
                for i in range(KT):
                    for (dst, src) in ((x8h, xth), (x8l, xtl)):
                        nc.sync.dma_start(
                            out=dst[i][:, :, ic_ * 512:(ic_ + 1) * 512].bitcast(U8),
                            in_=src[i * 128:(i + 1) * 128, :]
                            .rearrange("p (s n) -> p s n", s=2)
                            [:, :, ic_ * 512:(ic_ + 1) * 512])

            def load_xhalf(i, h):
                # query-column half of both interleave slots, one DMA
                for (dst, src) in ((x8h, xth), (x8l, xtl)):
                    d = dst[i][:, :, h * 1024:(h + 1) * 1024].bitcast(U8)
                    s = src[i * 128:(i + 1) * 128, :] \
                        .rearrange("p (s n) -> p s n", s=2)[:, :, h * 1024:(h + 1) * 1024]
                    nc.sync.dma_start(out=d, in_=s)

            # Priority order against the serial DMA device + 650ns/DMA issue
            # tax: pair-0 weights, first half of x (enough for the first two
            # ic chunks), V weights (band-0 v_groups), rest of x, then the
            # remaining projection weights.
            for i in range(KT):  # pair-0 kq weights, prepacked: 8 small DMAs
                nc.sync.dma_start(out=_u8(w08h[i]),
                                  in_=w0h[i * 128:(i + 1) * 128, :])
                nc.sync.dma_start(out=_u8(w08l[i]),
                                  in_=w0l[i * 128:(i + 1) * 128, :])
            for i in range(KT):
                load_xhalf(i, 0)
            for i in range(KT):
                nc.sync.dma_start(out=_u8(v8h[i]),
                                  in_=wvh[i * 128:(i + 1) * 128, :])
                nc.sync.dma_start(out=_u8(v8l[i]),
                                  in_=wvl[i * 128:(i + 1) * 128, :])
            for i in range(KT):
                load_xhalf(i, 1)
            loadw(640, 1024)     # k weights, pairs 1-3
            loadw(128, 512)      # q weights, pairs 1-3
            for i in range(4):
                nc.sync.dma_start(out=wo_sb[i], in_=wo[i * 128:(i + 1) * 128, :])
            # ^ load completion order (serial DMA device):
            #   pair0 kq slices ~10us, x first half ~16us, v ~21us,
            #   x second half ~27us, remaining weights ~36us, wo ~38us
            for j in range(NJ):
                nc.vector.memset(vp[j][:, :, HD:HD + 1], 1.0)
            masks.make_identity(nc, ident)
            nc.vector.memset(zt260, 0.0)

            TERMS = ((w8h, x8h), (w8h, x8l), (w8l, x8h))

            # ---- projection helpers (fp8 DoubleRow, 3-term hi/lo) ----------
            def qk_wsel(is_k, hp2):
                # pair 0 reads the prepacked early-load tiles
                if hp2 == 0:
                    return w08h, w08l, (0 if is_k else 128)
                return w8h, w8l, 512 * is_k + 128 * hp2

            def qk_group(is_k, hp, ic):
                ps = mmp.tile([128, 512], F32, tag="mm512", name=f"qk{is_k}{hp}{ic}")
                whi, wlo, cb = qk_wsel(is_k, hp)
                first = True
                for kt in range(KT):
                    for (wa, xa) in ((whi, x8h), (whi, x8l), (wlo, x8h)):
                        nc.tensor.matmul(
                            ps,
                            wa[kt][:, :, cb:cb + 128],
                            xa[kt][:, :, ic * 512:(ic + 1) * 512],
                            start=first, stop=(kt == KT - 1 and wa is wlo),
                            perf_mode=DR)
                        first = False
                dst = kp[hp] if is_k else qp[hp]
                nc.vector.tensor_copy(
                    dst[0:64, 0, ic * 512:(ic + 1) * 512], ps[0:64, :])
                nc.vector.tensor_copy(
                    dst[0:64, 1, ic * 512:(ic + 1) * 512], ps[64:128, :])

            def v_group(j):
                ps = mmp.tile([128, 512], F32, tag="mm512", name=f"vg{j}")
                first = True
                for kt in range(KT):
                    for (wa, xa) in ((v8h, x8h), (v8l, x8h), (v8h, x8l)):
                        nc.tensor.matmul(
                            ps,
                            xa[kt][:, :, j * 128:(j + 1) * 128],
                            wa[kt],
                            start=first, stop=(kt == KT - 1 and wa is v8h and xa is x8l),
                            perf_mode=DR)
                        first = False
                nc.vector.tensor_copy(
                    vp[j][:, :, 0:HD],
                    ps.rearrange("p (h d) -> p h d", h=HPC))

            # pair hp's projection groups: k first (S^T j-loop needs all of
            # k), then q
            def pair_proj_units(hp):
                return ([(1, hp, ic) for ic in range(NIC)] +
                        [(0, hp, ic) for ic in range(NIC)])

            # ---- softmax exp on ACT (logit scale folded into exp) ----------
            def exp_act(ut, ps):
                nc.scalar.activation(out=ut, in_=ps, func=FT.Exp, scale=EXP_SCALE)

            # ---- transposed P@V: one key-tile round of 8 accumulations -----
            # The 4 qt accumulation groups share one PSUM bank; a start=True
            # there wipes the whole bank (sibling groups lose their first
            # contribution), so the bank is zeroed once by a PE matmul
            # against a zero rhs and every accumulation runs start=False.
            def pv_alloc(hp, ic):
                pv2 = [pvp.tile([128, NIC, HD + 1], F32, tag="pv",
                                name=f"pv{hp}{ic}{h}") for h in range(2)]
                for h01 in range(2):
                    nc.tensor.matmul(
                        pv2[h01].rearrange("p a b -> p (a b)"),
                        ident, zt260,
                        start=True, stop=True)
                return pv2

            def pvt_kt(hp, uts, pv2, kt):
                for h01 in range(2):
                    for qt in range(4):
                        nc.tensor.matmul(
                            pv2[h01][:, qt, :],
                            uts[kt][:, 512 * h01 + 128 * qt:512 * h01 + 128 * (qt + 1)],
                            vp[kt][:, 2 * hp + h01, :],
                            start=False, stop=(kt == NJ - 1),
                            skip_group_check=True)

            # ---- softmax normalize: denominators -> 4 st2 tiles ------------
            def norm_band(hp, ic, pv2):
                rts = []
                for h01 in range(2):
                    rt = rtp.tile([128, NIC], F32, tag="rt", name=f"rt{hp}{ic}{h01}")
                    nc.vector.reciprocal(
                        rt, pv2[h01][:, :, HD:HD + 1].rearrange("p a b -> p (a b)"))
                    rts.append(rt)
                st2s = []
                for qt in range(4):
                    st2 = st2p.tile([128, 128], BF16, tag="st2", name=f"s2{hp}{ic}{qt}")
                    for h01 in range(2):
                        nc.vector.tensor_scalar(
                            st2[:, 64 * h01:64 * h01 + 64],
                            pv2[h01][:, qt, 0:HD],
                            rts[h01][:, qt:qt + 1], None, MUL)
                    st2s.append(st2)
                if dbg and hp == 0 and ic == 0:
                    ds = dbgp.tile([128, 128], F32, tag="dst2")
                    nc.vector.tensor_copy(ds, st2s[0])
                    nc.sync.dma_start(out=d_st2[:, :], in_=ds)
                    dr = dbgp.tile([128, 4], F32, tag="drt")
                    nc.vector.tensor_copy(dr, rts[0])
                    nc.sync.dma_start(out=d_rt[:, :], in_=dr)
                return st2s

            def transpose_unit(hp, ic, qt, st2):
                tp = mmp.tile([128, 512], F32, tag="mm512", name=f"tp{hp}{ic}{qt}")
                tpb = tp[:, 0:64].bitcast(BF16)
                nc.tensor.transpose(tpb, st2, ident)
                nc.vector.tensor_copy(
                    ot[hp][:, ic * 512 + 128 * qt:ic * 512 + 128 * (qt + 1)], tpb)

            # ---- final out-projection (bf16), one [128,512] group ----------
            def out_group(it, oc):
                ps = mmp.tile([128, 512], F32, tag="mm512",
                              name=f"psf{it}{oc}")
                for ktt in range(4):
                    nc.tensor.matmul(
                        ps,
                        ot[ktt][:, it * 128:(it + 1) * 128],
                        wo_sb[ktt][:, oc * 512:(oc + 1) * 512],
                        start=(ktt == 0), stop=(ktt == 3))
                fs = fsp.tile([128, 512], BF16, tag="fs", name=f"fs{it}{oc}")
                nc.vector.tensor_copy(fs, ps)
                nc.sync.dma_start(
                    out=out[it * 128:(it + 1) * 128, oc * 512:(oc + 1) * 512],
                    in_=fs)

            # ---- band pipeline ---------------------------------------------
            # Slot-based software pipeline, one slot per key-tile j. The ACT
            # exp stream is the critical path: every slot leads with its S^T
            # (so ACT never starves) followed by ~1us of ready PE work:
            #   - P@V of band n-1, key-tile j (one per slot, lag-16)
            #   - a flexible queue (transposes of n-1, out-projection
            #     half-groups of n-1, next pair's projection half-groups,
            #     band-0/1 V-projection halves), drained <=2 units/slot.
            # The last band chases its own exp stream at lag 2 instead so
            # the tail stays short.
            def qk_half(is_k, hp2, ic2, phase):
                # half of a 12-matmul projection group; phase 1 finishes the
                # accumulation and drains PSUM to the fp8 q/k tiles
                if phase == 0:
                    ps = mmp.tile([128, 512], F32, tag="mm512",
                                  name=f"qk{is_k}{hp2}{ic2}")
                    qk_half.ps = ps
                else:
                    ps = qk_half.ps
                whi, wlo, cb = qk_wsel(is_k, hp2)
                terms = ((whi, x8h), (whi, x8l), (wlo, x8h))
                seq = [(kt, t) for kt in range(KT) for t in range(3)]
                half = seq[:6] if phase == 0 else seq[6:]
                for (kt, t) in half:
                    wa, xa = terms[t]
                    nc.tensor.matmul(
                        ps,
                        wa[kt][:, :, cb:cb + 128],
                        xa[kt][:, :, ic2 * 512:(ic2 + 1) * 512],
                        start=(phase == 0 and (kt, t) == half[0]),
                        stop=(phase == 1 and (kt, t) == half[-1]),
                        perf_mode=DR)
                if phase == 1:
                    dst = kp[hp2] if is_k else qp[hp2]
                    nc.vector.tensor_copy(
                        dst[0:64, 0, ic2 * 512:(ic2 + 1) * 512], ps[0:64, :])
                    nc.vector.tensor_copy(
                        dst[0:64, 1, ic2 * 512:(ic2 + 1) * 512], ps[64:128, :])

            def v_half(j, phase):
                if phase == 0:
                    ps = mmp.tile([128, 512], F32, tag="mm512", name=f"vg{j}")
                    v_half.ps = ps
                else:
                    ps = v_half.ps
                seq = [(kt, t) for kt in range(KT) for t in range(3)]
                half = seq[:6] if phase == 0 else seq[6:]
                VT = ((v8h, x8h), (v8l, x8h), (v8h, x8l))
                for (kt, t) in half:
                    wa, xa = VT[t]
                    nc.tensor.matmul(
                        ps,
                        xa[kt][:, :, j * 128:(j + 1) * 128],
                        wa[kt],
                        start=(phase == 0 and (kt, t) == half[0]),
                        stop=(phase == 1 and (kt, t) == half[-1]),
                        perf_mode=DR)
                if phase == 1:
                    nc.vector.tensor_copy(
                        vp[j][:, :, 0:HD],
                        ps.rearrange("p (h d) -> p h d", h=HPC))

            def out_half(it, oc, phase):
                if phase == 0:
                    ps = mmp.tile([128, 512], F32, tag="mm512",
                                  name=f"psf{it}{oc}")
                    out_half.ps = ps
                else:
                    ps = out_half.ps
                rng = (0, 1) if phase == 0 else (2, 3)
                for ktt in rng:
                    nc.tensor.matmul(
                        ps,
                        ot[ktt][:, it * 128:(it + 1) * 128],
                        wo_sb[ktt][:, oc * 512:(oc + 1) * 512],
                        start=(ktt == 0), stop=(ktt == 3))
                if phase == 1:
                    fs = fsp.tile([128, 512], BF16, tag="fs",
                                  name=f"fs{it}{oc}")
                    nc.vector.tensor_copy(fs, ps)
                    nc.sync.dma_start(
                        out=out[it * 128:(it + 1) * 128,
                                oc * 512:(oc + 1) * 512],
                        in_=fs)

            # prologue: only the ic0 projections of pair 0 (the rest of
            # pair 0 is flex work inside band 0, ordered by S^T deadline
            # and by when its x/v DMA slices land)
            qk_group(1, 0, 0)
            qk_group(0, 0, 0)

            bands = [(hp, ic) for hp in range(NPAIRS) for ic in range(NIC)]
            NB = len(bands)

            def done_units(dhp, dic, dst2s):
                us = []
                for qt in range(4):
                    us.append((lambda qt=qt, dhp=dhp, dic=dic, s=dst2s[qt]:
                               transpose_unit(dhp, dic, qt, s)))
                if dhp == NPAIRS - 1:
                    for k in range(8):
                        it, oc = 4 * dic + k // 2, k % 2
                        us.append((lambda it=it, oc=oc: out_half(it, oc, 0)))
                        us.append((lambda it=it, oc=oc: out_half(it, oc, 1)))
                return us

            # Pipeline state:
            #  prev    = band n-1 (uts + pv accumulators for band-1 carry)
            #  pending = bands whose last pv key-tiles + normalize run in
            #            the next band's slots 0/1
            #  done_q  = normalized bands awaiting transpose + out-proj,
            #            drained lazily through the flex queue
            prev = None
            pending = []
            done_q = []
            for n, (hp, ic) in enumerate(bands):
                nxt = pair_proj_units(hp + 1) if hp + 1 < NPAIRS else []

                def add_qk(fx, units):
                    for u in units:
                        fx.append((lambda u=u: qk_half(*u, 0)))
                        fx.append((lambda u=u: qk_half(*u, 1)))

                def add_v(fx, j2s):
                    for j2 in j2s:
                        fx.append((lambda j2=j2: v_half(j2, 0)))
                        fx.append((lambda j2=j2: v_half(j2, 1)))

                flex = []
                if hp == 0 and ic == 0:
                    # rest of pair 0, interleaved with the first V groups in
                    # DMA-arrival/deadline order (k ic2/3 wait on x 2nd half)
                    add_qk(flex, [(1, 0, 1), (0, 0, 1)])
                    add_v(flex, range(0, 4))
                    add_qk(flex, [(1, 0, 2)])
                    add_v(flex, range(4, 6))
                    add_qk(flex, [(0, 0, 2), (1, 0, 3)])
                    add_v(flex, range(6, 8))
                    add_qk(flex, [(0, 0, 3)])
                elif hp == 0 and ic == 1:
                    add_v(flex, range(8, NJ))
                elif hp == 0 and ic == 2:
                    add_qk(flex, nxt[0:6])
                elif hp == 0 and ic == 3:
                    add_qk(flex, nxt[6:8])
                else:
                    add_qk(flex, nxt[2 * ic:2 * ic + 2])

                uts = []
                pv2 = None
                for j in range(NJ):
                    ps = expp.tile([128, 1024], F32, tag="exps",
                                   name=f"se{hp}{ic}{j}")
                    for h01 in range(2):
                        base = 32 * h01
                        nc.tensor.matmul(
                            ps[:, 512 * h01:512 * (h01 + 1)],
                            kp[hp][base:base + 32, :, j * 128:(j + 1) * 128],
                            qp[hp][base:base + 32, :, ic * 512:(ic + 1) * 512],
                            start=True, stop=True, perf_mode=DR)
                    ut = utp.tile([128, 1024], BF16, tag="ut",
                                  name=f"ut{hp}{ic}{j}")
                    exp_act(ut, ps)
                    uts.append(ut)
                    if dbg and n == 0 and j == 0:
                        dut = dbgp.tile([128, 1024], F32, tag="dut")
                        nc.vector.tensor_copy(dut, ut)
                        nc.sync.dma_start(out=d_ut[:, :], in_=dut)

                    # previous bands' last pv key-tiles + normalize
                    if j <= 1 and pending:
                        work = 2
                        while pending and work > 0:
                            ent = pending[0]
                            while ent["kts"] and work > 0:
                                pvt_kt(ent["hp"], ent["uts"], ent["pv2"],
                                       ent["kts"].pop(0))
                                work -= 1
                            if not ent["kts"]:
                                done_q.append(
                                    (ent["hp"], ent["ic"],
                                     norm_band(ent["hp"], ent["ic"],
                                               ent["pv2"])))
                                pending.pop(0)
                    # bands 1..12 carry the previous band's P@V, one key
                    # tile per slot (the previous band's exps are all done)
                    if 1 <= n <= 12:
                        php, pic, puts, ppv2 = prev
                        pvt_kt(php, puts, ppv2, j)
                    # hp==3 bands self-chase their own P@V at lag 2 so the
                    # final out-projections distribute into bands 13-15
                    if n >= 12 and 2 <= j:
                        if pv2 is None:
                            pv2 = pv_alloc(hp, ic)
                        pvt_kt(hp, uts, pv2, j - 2)

                    budget = 2
                    while budget > 0:
                        if not flex and done_q:
                            flex.extend(done_units(*done_q.pop(0)))
                        if not flex:
                            break
                        flex.pop(0)()
                        budget -= 1
                while flex:
                    flex.pop(0)()

                if 1 <= n <= 12:
                    # prev's P@V fully accumulated here; normalize next band
                    php, pic, puts, ppv2 = prev
                    pending.append({"hp": php, "ic": pic, "uts": puts,
                                    "pv2": ppv2, "kts": []})
                if n >= 12:
                    pending.append({"hp": hp, "ic": ic, "uts": uts,
                                    "pv2": pv2, "kts": [14, 15]})
                if n < 12:
                    # carried bands get their pv accumulators at band end
                    pv2 = pv_alloc(hp, ic)
                prev = (hp, ic, uts, pv2)

            if dbg:
                nc.sync.dma_start(out=d_qp[:, :], in_=_u8(qp[0]))
                nc.sync.dma_start(out=d_kp[:, :], in_=_u8(kp[0]))
                dvp = dbgp.tile([128, HPC * (HD + 1)], F32, tag="dvp")
                nc.vector.tensor_copy(
                    dvp, vp[0].rearrange("p h d -> p (h d)"))
                nc.sync.dma_start(out=d_vp[:, :], in_=dvp)
                dot = dbgp.tile([128, N], F32, tag="dot")
                nc.vector.tensor_copy(dot, ot[0])
                nc.sync.dma_start(out=d_ot[:, :], in_=dot)

            # tail: finish pending pv + norms, then drain done_q
            for ent in pending:
                for kt in ent["kts"]:
                    pvt_kt(ent["hp"], ent["uts"], ent["pv2"], kt)
                done_q.append((ent["hp"], ent["ic"],
                               norm_band(ent["hp"], ent["ic"], ent["pv2"])))
            while done_q:
                for u in done_units(*done_q.pop(0)):
                    u()

    return nc


# ---------------------------------------------------------------------------
# Cached SPMD runner (replicates bass2jax.run_bass_via_pjrt's multi-core path
# but jits once so repeated calls don't recompile).
# ---------------------------------------------------------------------------
_RUNNER = None


def _build_runner():
    nc = build_nc()
    bass2jax.install_neuronx_cc_hook()

    partition_name = (nc.partition_id_tensor.name
                      if nc.partition_id_tensor else None)
    in_names, out_names, out_avals, zero_shapes = [], [], [], []
    for alloc in nc.m.functions[0].allocations:
        if not isinstance(alloc, mybir.MemoryLocationSet):
            continue
        name = alloc.memorylocations[0].name
        if alloc.kind == "ExternalInput":
            if name != partition_name:
                in_names.append(name)
        elif alloc.kind == "ExternalOutput":
            shape = tuple(alloc.tensor_shape)
            dtype = mybir.dt.np(alloc.dtype)
            out_names.append(name)
            out_avals.append(jax.core.ShapedArray(shape, dtype))
            zero_shapes.append((shape, dtype))
    n_params = len(in_names)
    n_outs = len(out_avals)
    all_in_names = list(in_names) + list(out_names)
    if partition_name is not None:
        all_in_names.append(partition_name)

    def _body(*args):
        operands = list(args)
        if partition_name is not None:
            operands.append(bass2jax.partition_id_tensor())
        outs = bass2jax._bass_exec_p.bind(
            *operands,
            out_avals=tuple(out_avals),
            in_names=tuple(all_in_names),
            out_names=tuple(out_names),
            lowering_input_output_aliases=(),
            sim_require_finite=True,
            sim_require_nnan=True,
            nc=nc,
        )
        return tuple(outs)

    devices = jax.devices()[:NCORES]
    mesh = Mesh(np.asarray(devices), ("core",))
    in_specs = (PartitionSpec("core"),) * (n_params + n_outs)
    out_specs = (PartitionSpec("core"),) * n_outs
    donate = tuple(range(n_params, n_params + n_outs))
    sharded = jax.jit(
        shard_map(_body, mesh=mesh, in_specs=in_specs, out_specs=out_specs,
                  check_rep=False),
        donate_argnums=donate, keep_unused=True)

    def run(in_maps):
        concat_in = [
            np.concatenate([np.asarray(in_maps[c][nm]) for c in range(NCORES)],
                           axis=0)
            for nm in in_names
        ]
        concat_zeros = [np.zeros((NCORES * s[0], *s[1:]), dt)
                        for (s, dt) in zero_shapes]
        out_arrs = sharded(*concat_in, *concat_zeros)
        out_arrs = [np.asarray(a) for a in out_arrs]
        return [
            {nm: out_arrs[i].reshape(NCORES, *out_avals[i].shape)[c]
             for i, nm in enumerate(out_names)}
            for c in range(NCORES)
        ]

    return run


F8NP = ml_dtypes.float8_e4m3


def _hilo(a):
    hi = a.astype(F8NP)
    lo = (a - hi.astype(np.float32)).astype(F8NP)
    return hi, lo


def _dr_rows(a):
    """[D, C] -> DoubleRow row grouping [4*128, 2*C] (d = 256*kt+128*s+p)."""
    d, c = a.shape
    return np.ascontiguousarray(
        a.reshape(4, 2, 128, c).transpose(0, 2, 1, 3).reshape(512, 2 * c))


def _prep_inputs(x, w_qkv, w_out):
    """Host-side shard prep: per-core fp8 hi/lo DR operands + bf16 wo."""
    x = np.asarray(x, dtype=np.float32)
    w_qkv = np.asarray(w_qkv, dtype=np.float32)
    w_out = np.asarray(w_out, dtype=np.float32)

    w3 = w_qkv.reshape(D, 3, H, HD)
    wq_all, wk_all, wv_all = w3[:, 0], w3[:, 1], w3[:, 2]
    wo_h = w_out.reshape(H, HD, D)

    in_maps = []
    for c in range(NCORES):
        b, g = divmod(c, 2)
        hs = slice(8 * g, 8 * g + 8)
        xt = np.ascontiguousarray(x[b].T)                      # [D, N]
        wq = wq_all[:, hs]                                     # [D, 8, 64]
        wk = wk_all[:, hs]
        wv = wv_all[:, hs].reshape(D, 512) * WSCALE

        # qk weight columns: per (is_k, hp): [A d0:32 | B d0:32 | A d32:64 |
        # B d32:64] with A = head 2hp, B = 2hp+1 (no logit scale: folded
        # into exp)
        wqk = np.empty((D, 2, NPAIRS, 2, 2, 32), np.float32)
        for is_k, w_ in ((0, wq), (1, wk)):
            for hp in range(NPAIRS):
                for sh in range(2):
                    for ab in range(2):
                        wqk[:, is_k, hp, sh, ab, :] = \
                            w_[:, 2 * hp + ab, 32 * sh:32 * sh + 32]
        wqk = wqk.reshape(D, 1024) * WSCALE
        # pair-0 k/q columns prepacked for the early prologue load
        w0 = np.concatenate([wqk[:, 512:640], wqk[:, 0:128]], axis=1)

        xh, xl = _hilo(xt)
        qh, ql = _hilo(wqk)
        w0h_, w0l_ = _hilo(w0)
        vh, vl = _hilo(wv)
        wo_c = (wo_h[hs].reshape(512, D) / WSCALE).astype(ml_dtypes.bfloat16)

        def u8(a):
            return _dr_rows(a.astype(np.float32)).astype(F8NP).view(np.uint8)

        in_maps.append({
            "xth": u8(xh), "xtl": u8(xl),
            "wqh": u8(qh), "wql": u8(ql),
            "w0h": u8(w0h_), "w0l": u8(w0l_),
            "wvh": u8(vh), "wvl": u8(vl),
            "wo": wo_c,
        })
    return in_maps


def get_runner():
    global _RUNNER
    if _RUNNER is None:
        _RUNNER = _build_runner()
    return _RUNNER


def kernel(x, w_qkv, w_out, b_out):
    b_out = np.asarray(b_out, dtype=np.float32)
    in_maps = _prep_inputs(x, w_qkv, w_out)
    results = get_runner()(in_maps)
    out = np.empty((B, N, D), dtype=np.float32)
    for b in range(B):
        out[b] = (results[2 * b]["out"].astype(np.float32)
                  + results[2 * b + 1]["out"].astype(np.float32) + b_out)
    return out


# revision 60
# speedup vs baseline: 2.2040x; 2.2040x over previous
"""Multi-head attention block (dense transformer) on 8 trn2 NeuronCores.

Sharding: batch (4) x head-group (2 groups of 8 heads) = 8 cores. Each core
computes, for its batch b and its 8 heads:
    qkv slice -> per-head softmax(q k^T / sqrt(D)) v -> partial out proj.
Host sums the two head-group partials per batch and adds the output bias.

Device dataflow is fully "transposed": the projection produces qT/kT with
head-dim on partitions (what the S^T matmul wants) and V in natural layout
with a fused ones-column, so P @ V also yields the softmax denominators.
exp() runs on the scalar engine straight out of PSUM in [128, 1024] windows.
No max-subtraction: logits are small by construction, exp is safe.

v3: the q/k projection runs as fp8e4 DoubleRow matmuls (4 matmuls of
K_eff=256 instead of 8 bf16 matmuls of K=128), halving that phase's PE
instruction count and cycles. Host supplies x and the q/k weights as fp8
with the weights scaled x32 into fp8's precision sweet spot (w_qkv has
sigma 1/32, subnormal in fp8 otherwise); the combined 1/sqrt(D)/32^2
logit scale folds into the exp's scale operand. The real backend's cost
tracks total instruction count (~86-95 ns/instruction), so fewer, larger
matmuls win over the cost model's cycle counts.
"""

import numpy as np
import ml_dtypes
import jax
import jax.core
from jax.experimental.shard_map import shard_map
from jax.sharding import Mesh, PartitionSpec

import concourse.bass as bass
import concourse.mybir as mybir
import concourse.tile as tile
import concourse.bass2jax as bass2jax
from concourse.vector_clock import ScopedClock

# ---------------------------------------------------------------------------
# Workaround for the pinned walrus compiler: it rejects instructions carrying
# more than one sync wait. Split extra waits onto NOPs inserted immediately
# before the instruction in the same engine stream (identical semantics: the
# engine blocks on each wait in turn).
# ---------------------------------------------------------------------------
_MAX_WAITS = 1
_patched = False


def _split_waits(ordered):
    for bb_name, insts in ordered.items():
        out = []
        for inst in insts:
            si = inst.sync_info
            waits = list(si.on_wait) if si and si.on_wait else []
            if len(waits) > _MAX_WAITS:
                rest, keep = waits[:-_MAX_WAITS], waits[-_MAX_WAITS:]
                for k in range(0, len(rest), _MAX_WAITS):
                    out.append(mybir.InstNoOp(
                        name=f"{inst.name}-wsplit{k}",
                        sync_info=mybir.SyncInfo(
                            on_wait=rest[k:k + _MAX_WAITS], on_update=[]),
                        bass_nofuse=True,
                        engine=inst.engine,
                    ))
                inst.sync_info = mybir.SyncInfo(
                    on_wait=keep, on_update=list(si.on_update or []))
            out.append(inst)
        ordered[bb_name] = out
    return ordered


def _install_patches():
    global _patched
    if _patched:
        return
    _patched = True

    orig_lower = tile.TileContext._lower_ordered_insts

    def lower_with_split(self, ordered):
        return orig_lower(self, _split_waits(ordered))

    tile.TileContext._lower_ordered_insts = lower_with_split

    def drain_and_barrier(self, tick_clock, wait_clock):
        nc = self.nc
        drain_inst = nc.sync.drain()
        wait_clock.add_sem_waits(
            drain_inst.ins, ScopedClock({None: tick_clock.global_clock}))
        si = drain_inst.ins.sync_info
        waits = list(si.on_wait) if si and si.on_wait else []
        upds = list(si.on_update) if si and si.on_update else []
        if len(waits) > _MAX_WAITS:
            drain_inst.ins.sync_info = mybir.SyncInfo(
                on_wait=waits[:_MAX_WAITS], on_update=upds)
            for i in range(_MAX_WAITS, len(waits), _MAX_WAITS):
                nop = nc.sync.nop()
                nop.ins.sync_info = mybir.SyncInfo(
                    on_wait=waits[i:i + _MAX_WAITS], on_update=[])
        nc.all_engine_barrier()
        assert self.sems is not None
        popped = nc._tile_sem_poison_stack.pop()
        assert popped is self._sem_poison
        nc.clear_and_free_semaphores(list(self.sems.allocated().values()))
        nc.all_engine_barrier()

    tile.TileContext._drain_and_barrier = drain_and_barrier


# ---------------------------------------------------------------------------
# Problem constants (hardcoded per the task contract).
# ---------------------------------------------------------------------------
B, N, D, H, HD = 4, 2048, 1024, 16, 64
NCORES = 8
HPC = 8                 # heads per core
NPAIRS = HPC // 2       # head pairs per core
KD = D // 128           # 8 contraction tiles for the projections
NJ = N // 128           # 16 key tiles
NIC = N // 512          # 4 query chunks of 512
NT = N // 128           # 16 output row tiles
SCALE = float(D) ** -0.5

BF16 = mybir.dt.bfloat16
F32 = mybir.dt.float32
F8 = mybir.dt.float8e4
U8 = mybir.dt.uint8
FT = mybir.ActivationFunctionType
DR = mybir.MatmulPerfMode.DoubleRow
KT4 = 4                  # DoubleRow contraction tiles (256 dims) over D
# q/k weights are scaled x32 on the host (fp8 range, sigma~1); the logit
# scale and the 1/32^2 compensation fold into the exp:
EXP_SCALE = SCALE / 1024.0


def _u8(ap):
    return ap.bitcast(U8).rearrange("p a b -> p (a b)")


def build_nc(loop_n: int = 1, exp_split: bool = False, no_exp: bool = False,
             st_k128: bool = False, spread_proj: bool = True) -> bass.Bass:
    """loop_n > 1 wraps the whole body in a hardware loop (benchmark builds
    only) so per-iteration device time can be extracted from wall clock."""
    _install_patches()
    nc = bass.Bass()

    xt = nc.dram_tensor("xt", [D, N], BF16, kind="ExternalInput")
    x8d = nc.dram_tensor("x8d", [512, 2 * N], U8, kind="ExternalInput")
    wq8d = nc.dram_tensor("wq8d", [512, 2 * 1024], U8, kind="ExternalInput")
    wv = nc.dram_tensor("wv", [D, 512], BF16, kind="ExternalInput")
    wo = nc.dram_tensor("wo", [512, D], BF16, kind="ExternalInput")
    out = nc.dram_tensor("out", [N, D], F32, kind="ExternalOutput")
    # per-(head, i-chunk) softmax denominator rows, bounced through DRAM to
    # broadcast across partitions
    rsums = nc.dram_tensor("rsums", [HPC * NIC, 512], F32, kind="Internal")

    import contextlib

    with tile.TileContext(nc) as tc:
        loop_ctx = (tc.For_i(0, loop_n, 1,
                             hint_engines=(mybir.EngineType.PE,
                                           mybir.EngineType.Activation,
                                           mybir.EngineType.DVE,
                                           mybir.EngineType.SP))
                    if loop_n > 1 else contextlib.nullcontext())
        with (
            loop_ctx,
            tc.tile_pool(name="persist", bufs=1) as pers,
            tc.tile_pool(name="expp", bufs=2, space="PSUM") as expp,
            tc.tile_pool(name="pvp", bufs=2, space="PSUM") as pvp,
            tc.tile_pool(name="mmp", bufs=2, space="PSUM") as mmp,
            tc.tile_pool(name="utp", bufs=20) as utp,
            tc.tile_pool(name="pvstage", bufs=6) as pvstage,
            tc.tile_pool(name="rp", bufs=4) as rp,
            tc.tile_pool(name="fstage", bufs=4) as fstage,
        ):
            # ---- persistent SBUF tensors -----------------------------------
            xt_sb = [pers.tile([128, N], BF16, tag=f"xt{i}", name=f"xt{i}") for i in range(KD)]
            w8_sb = [pers.tile([128, 2, 1024], F8, tag=f"w8{i}", name=f"w8{i}") for i in range(KT4)]
            x8_sb = [pers.tile([128, 2, N], F8, tag=f"x8{i}", name=f"x8{i}") for i in range(KT4)]
            wv_sb = [pers.tile([128, 512], BF16, tag=f"wv{i}", name=f"wv{i}") for i in range(KD)]
            wo_sb = [pers.tile([128, D], BF16, tag=f"wo{i}", name=f"wo{i}") for i in range(4)]
            qkT_sb = [pers.tile([128, N], BF16, tag=f"qk{i}", name=f"qk{i}") for i in range(8)]
            vp_sb = [pers.tile([128, HPC, HD + 1], BF16, tag=f"vp{i}", name=f"vp{i}")
                     for i in range(NJ)]
            ot_sb = [pers.tile([128, N], BF16, tag=f"ot{i}", name=f"ot{i}") for i in range(NPAIRS)]

            for i in range(KT4):
                nc.sync.dma_start(out=_u8(w8_sb[i]), in_=wq8d[i * 128:(i + 1) * 128, :])
            for i in range(KT4):
                nc.sync.dma_start(out=_u8(x8_sb[i]), in_=x8d[i * 128:(i + 1) * 128, :])
            for i in range(KD):
                nc.sync.dma_start(out=xt_sb[i], in_=xt[i * 128:(i + 1) * 128, :])
            for i in range(KD):
                nc.sync.dma_start(out=wv_sb[i], in_=wv[i * 128:(i + 1) * 128, :])
            for i in range(4):
                nc.sync.dma_start(out=wo_sb[i], in_=wo[i * 128:(i + 1) * 128, :])
            for j in range(NJ):
                nc.vector.memset(vp_sb[j][:, :, HD:HD + 1], 1.0)

            # ---- stage A helpers -------------------------------------------
            def project_v():
                # V natural layout; emitted after the first exp stream is
                # underway so ACT ramps up as early as possible.
                for j in range(NJ):
                    ps = mmp.tile([128, 512], F32, tag="mm512", name=f"psv{j}")
                    for kd in range(KD):
                        nc.tensor.matmul(
                            ps,
                            xt_sb[kd][:, j * 128:(j + 1) * 128],
                            wv_sb[kd],
                            start=(kd == 0), stop=(kd == KD - 1))
                    nc.vector.tensor_copy(
                        vp_sb[j][:, :, 0:HD],
                        ps.rearrange("p (h d) -> p h d", h=HPC))
            def project_group(ct, ic):
                # fp8 DoubleRow: 4 matmuls of K_eff=256 instead of 8 bf16
                ps = mmp.tile([128, 512], F32, tag="mm512", name=f"psq{ct}{ic}")
                for kt in range(KT4):
                    nc.tensor.matmul(
                        ps,
                        w8_sb[kt][:, :, ct * 128:(ct + 1) * 128],
                        x8_sb[kt][:, :, ic * 512:(ic + 1) * 512],
                        start=(kt == 0), stop=(kt == KT4 - 1),
                        perf_mode=DR)
                nc.vector.tensor_copy(
                    qkT_sb[ct][:, ic * 512:(ic + 1) * 512], ps)

            # projection work units for pair hp: kT first (S^T j-loop needs
            # all of kT), then qT
            def pair_proj_units(hp):
                return [(4 + hp, ic) for ic in range(NIC)] +                        [(hp, ic) for ic in range(NIC)]

            # ---- per-pair pipeline.  Pair hp+1's projection groups are
            # ---- spread across pair hp's exp phases so PE has filler work
            # ---- while ACT streams. ----------------------------------------
            for ct, icg in pair_proj_units(0):
                project_group(ct, icg)
            for hp in range(NPAIRS):
                if not spread_proj and hp + 1 < NPAIRS:
                    for ct, icg in pair_proj_units(hp + 1):
                        project_group(ct, icg)
                kT = qkT_sb[4 + hp]
                qT = qkT_sb[hp]
                nxt = pair_proj_units(hp + 1) if hp + 1 < NPAIRS else []
                for ic in range(NIC):
                    qsA = qT[0:64, ic * 512:(ic + 1) * 512]
                    qsB = qT[64:128, ic * 512:(ic + 1) * 512]
                    uts = []
                    for j in range(NJ):
                        ps = expp.tile([128, 1024], F32, tag="exps", name=f"se{hp}{ic}{j}")
                        if st_k128:
                            # timing-only: one K=128 matmul instead of the
                            # K=64 pair (wrong numerics, half the mm count)
                            nc.tensor.matmul(
                                ps[:, 0:512],
                                kT[:, j * 128:(j + 1) * 128],
                                qT[:, ic * 512:(ic + 1) * 512],
                                start=True, stop=True)
                            nc.tensor.matmul(
                                ps[:, 512:1024],
                                kT[:, j * 128:(j + 1) * 128],
                                qT[:, ic * 512:(ic + 1) * 512],
                                start=True, stop=True)
                        else:
                            nc.tensor.matmul(
                                ps[:, 0:512],
                                kT[0:64, j * 128:(j + 1) * 128], qsA,
                                start=True, stop=True)
                            nc.tensor.matmul(
                                ps[:, 512:1024],
                                kT[64:128, j * 128:(j + 1) * 128], qsB,
                                start=True, stop=True, tile_position=(64, 0))
                        ut = utp.tile([128, 1024], BF16, tag="ut", name=f"ut{hp}{ic}{j}")
                        if no_exp:
                            # timing-only variant: unload ACT entirely
                            nc.vector.tensor_copy(ut, ps)
                        elif exp_split:
                            nc.scalar.activation(out=ut[:, 0:512],
                                                 in_=ps[:, 0:512], func=FT.Exp,
                                                 scale=EXP_SCALE)
                            nc.scalar.activation(out=ut[:, 512:1024],
                                                 in_=ps[:, 512:1024], func=FT.Exp,
                                                 scale=EXP_SCALE)
                        else:
                            nc.scalar.activation(out=ut, in_=ps, func=FT.Exp,
                                                 scale=EXP_SCALE)
                        uts.append(ut)
                    if hp == 0 and ic == 0:
                        project_v()
                    elif ic >= 1 and spread_proj:
                        # 3/3/2 projection groups of the next pair
                        share = nxt[3 * (ic - 1):3 * ic] if ic < 3 else nxt[6:]
                        for ct, icg in share:
                            project_group(ct, icg)
                    for hh in range(2):
                        hloc = 2 * hp + hh
                        c0 = 512 * hh
                        pvt = pvp.tile([HD + 1, 512], F32, tag="pv", name=f"pv{hloc}{ic}")
                        for j in range(NJ):
                            nc.tensor.matmul(
                                pvt,
                                vp_sb[j][:, hloc, :],
                                uts[j][:, c0:c0 + 512],
                                start=(j == 0), stop=(j == NJ - 1))
                        stg = pvstage.tile([HD + 1, 512], F32, tag="pvs", name=f"st{hloc}{ic}")
                        nc.vector.tensor_copy(stg, pvt)
                        hic = hloc * NIC + ic
                        nc.sync.dma_start(out=rsums[hic:hic + 1, :],
                                          in_=stg[HD:HD + 1, :])
                        rt = rp.tile([HD, 512], F32, tag="rt", name=f"rt{hloc}{ic}")
                        srcap = rsums[hic:hic + 1, :]
                        nc.sync.dma_start(out=rt, in_=bass.AP(
                            tensor=srcap.tensor, offset=srcap.offset,
                            ap=[[0, HD]] + list(srcap.ap[1:])))
                        nc.vector.reciprocal(rt, rt)
                        nc.vector.tensor_mul(
                            ot_sb[hp][64 * hh:64 * hh + 64,
                                      ic * 512:(ic + 1) * 512],
                            stg[0:HD, :], rt)

                    if hp == NPAIRS - 1:
                        # all pairs have this i-chunk done: project it out
                        for it in range(4 * ic, 4 * ic + 4):
                            for oc in range(2):
                                ps = mmp.tile([128, 512], F32, tag="mm512",
                                              name=f"psf{it}{oc}")
                                for kt in range(4):
                                    nc.tensor.matmul(
                                        ps,
                                        ot_sb[kt][:, it * 128:(it + 1) * 128],
                                        wo_sb[kt][:, oc * 512:(oc + 1) * 512],
                                        start=(kt == 0), stop=(kt == 3))
                                fs = fstage.tile([128, 512], F32, tag="fs",
                                                 name=f"fs{it}{oc}")
                                nc.vector.tensor_copy(fs, ps)
                                nc.sync.dma_start(
                                    out=out[it * 128:(it + 1) * 128,
                                            oc * 512:(oc + 1) * 512],
                                    in_=fs)

    return nc


# ---------------------------------------------------------------------------
# Cached SPMD runner (replicates bass2jax.run_bass_via_pjrt's multi-core path
# but jits once so repeated calls don't recompile).
# ---------------------------------------------------------------------------
_RUNNER = None


def _build_runner():
    nc = build_nc()
    bass2jax.install_neuronx_cc_hook()

    partition_name = (nc.partition_id_tensor.name
                      if nc.partition_id_tensor else None)
    in_names, out_names, out_avals, zero_shapes = [], [], [], []
    for alloc in nc.m.functions[0].allocations:
        if not isinstance(alloc, mybir.MemoryLocationSet):
            continue
        name = alloc.memorylocations[0].name
        if alloc.kind == "ExternalInput":
            if name != partition_name:
                in_names.append(name)
        elif alloc.kind == "ExternalOutput":
            shape = tuple(alloc.tensor_shape)
            dtype = mybir.dt.np(alloc.dtype)
            out_names.append(name)
            out_avals.append(jax.core.ShapedArray(shape, dtype))
            zero_shapes.append((shape, dtype))
    n_params = len(in_names)
    n_outs = len(out_avals)
    all_in_names = list(in_names) + list(out_names)
    if partition_name is not None:
        all_in_names.append(partition_name)

    def _body(*args):
        operands = list(args)
        if partition_name is not None:
            operands.append(bass2jax.partition_id_tensor())
        outs = bass2jax._bass_exec_p.bind(
            *operands,
            out_avals=tuple(out_avals),
            in_names=tuple(all_in_names),
            out_names=tuple(out_names),
            lowering_input_output_aliases=(),
            sim_require_finite=True,
            sim_require_nnan=True,
            nc=nc,
        )
        return tuple(outs)

    devices = jax.devices()[:NCORES]
    mesh = Mesh(np.asarray(devices), ("core",))
    in_specs = (PartitionSpec("core"),) * (n_params + n_outs)
    out_specs = (PartitionSpec("core"),) * n_outs
    donate = tuple(range(n_params, n_params + n_outs))
    sharded = jax.jit(
        shard_map(_body, mesh=mesh, in_specs=in_specs, out_specs=out_specs,
                  check_rep=False),
        donate_argnums=donate, keep_unused=True)

    def run(in_maps):
        concat_in = [
            np.concatenate([np.asarray(in_maps[c][nm]) for c in range(NCORES)],
                           axis=0)
            for nm in in_names
        ]
        concat_zeros = [np.zeros((NCORES * s[0], *s[1:]), dt)
                        for (s, dt) in zero_shapes]
        out_arrs = sharded(*concat_in, *concat_zeros)
        out_arrs = [np.asarray(a) for a in out_arrs]
        return [
            {nm: out_arrs[i].reshape(NCORES, *out_avals[i].shape)[c]
             for i, nm in enumerate(out_names)}
            for c in range(NCORES)
        ]

    return run


F8NP = ml_dtypes.float8_e4m3


def _dr_rows(a):
    """[D, C] -> DoubleRow row grouping [4*128, 2*C] (d = 256*kt+128*s+p)."""
    d, c = a.shape
    return np.ascontiguousarray(
        a.reshape(4, 2, 128, c).transpose(0, 2, 1, 3).reshape(512, 2 * c))


def _prep_inputs(x, w_qkv, w_out):
    """Host-side shard prep: xt/wv/wo bf16 + fp8 DoubleRow x and qk weights
    (scaled x32 into fp8 range; the exp scale compensates)."""
    x = np.asarray(x, dtype=np.float32)
    w_qkv = np.asarray(w_qkv, dtype=np.float32)
    w_out = np.asarray(w_out, dtype=np.float32)

    w3 = w_qkv.reshape(D, 3, H, HD)
    wq, wk, wv_ = w3[:, 0], w3[:, 1], w3[:, 2]
    wo_h = w_out.reshape(H, HD, D)

    in_maps = []
    for c in range(NCORES):
        b, g = divmod(c, 2)
        hs = slice(8 * g, 8 * g + 8)
        xtf = np.ascontiguousarray(x[b].T)
        xt = xtf.astype(ml_dtypes.bfloat16)
        wqkf = np.concatenate([
            wq[:, hs].reshape(D, 512),
            wk[:, hs].reshape(D, 512),
        ], axis=1) * 32.0
        x8 = _dr_rows(xtf).astype(F8NP).view(np.uint8)
        wq8 = _dr_rows(wqkf).astype(F8NP).view(np.uint8)
        wv = wv_[:, hs].reshape(D, 512).astype(ml_dtypes.bfloat16)
        wo = wo_h[hs].reshape(512, D).astype(ml_dtypes.bfloat16)
        in_maps.append({"xt": xt, "x8d": x8, "wq8d": wq8, "wv": wv, "wo": wo})
    return in_maps


def get_runner():
    global _RUNNER
    if _RUNNER is None:
        _RUNNER = _build_runner()
    return _RUNNER


def kernel(x, w_qkv, w_out, b_out):
    b_out = np.asarray(b_out, dtype=np.float32)
    in_maps = _prep_inputs(x, w_qkv, w_out)
    results = get_runner()(in_maps)
    out = np.empty((B, N, D), dtype=np.float32)
    for b in range(B):
        out[b] = results[2 * b]["out"] + results[2 * b + 1]["out"] + b_out
    return out

